# revision 1
# baseline (speedup 1.0000x reference)
"""Trainium2 Bass kernel for nn_BaseSO3Convolution (SO(3) equivariant conv).

y[a,o,c] = sum_{e: idx_i[e]=a} sum_{paths (s1,s2,o)} CG[s1,s2,o] * dir[e,s1]
           * Wij[e, l(s1), c] * x[idx_j[e], s2, c]
with Wij = (radial @ Wf + bf) * cutoff.

Strategy (per core; 8 cores, edges sharded by receiving atom):
  - Edges sorted by idx_i, packed into 128-slot blocks whose atoms span < 14
    consecutive atoms.  For each block the entire CG contraction AND the
    within-block segment-sum are fused into 27 accumulating TensorE matmuls
    with contraction over the 128 edge slots:
        psum[(a mod 14)*9+o, c] += sum_e stat_q[e, (am,o)] * xjW[e, q, c]
    where q=(l1,s2) and stat_q[e, col] = CG-coef * dir placed at the edge's
    receiving-atom ring position (host-built, bf16).
  - xjW[e,(l1,s2),c] = Wij[e,l1,c] * xj[e,s2,c] is one broadcast DVE multiply.
  - xj gathered by indirect DMA from x (bf16) using idx_j.
  - Wij computed on-device by a small matmul (radialAug^T stationary).
  - Per-block partial sums land in a [128, NB*128] output; the host merges
    block partials into y (a few thousand 9x128 adds).
"""

import sys

sys.path.insert(0, "/opt/trn_rl_repo")

import numpy as np
import ml_dtypes
from math import factorial, sqrt

BF16 = ml_dtypes.bfloat16

LMAX = 2
SH = 9
N_ATOMS = 1000
N_EDGES = 10000
C = 128
NR = 20
RAUG = NR + 1  # radial basis augmented with cutoff column (bias folding)
NCORES = 8
BLK = 128  # edge slots per block
RING = 14  # atom ring size (mod-14 placement), 14*9=126 <= 128 psum partitions
NQ = 27  # (l1, s2) combos


# ---------------------------------------------------------------------------
# Clebsch-Gordan (real spherical harmonics) — self-contained copy
# ---------------------------------------------------------------------------
def _cg_complex(l1, m1, l2, m2, l3, m3):
    if m3 != m1 + m2 or not (abs(l1 - l2) <= l3 <= l1 + l2):
        return 0.0
    pre = sqrt(
        (2 * l3 + 1)
        * factorial(l3 + l1 - l2)
        * factorial(l3 - l1 + l2)
        * factorial(l1 + l2 - l3)
        / factorial(l1 + l2 + l3 + 1)
    )
    pre *= sqrt(
        factorial(l3 + m3)
        * factorial(l3 - m3)
        * factorial(l1 - m1)
        * factorial(l1 + m1)
        * factorial(l2 - m2)
        * factorial(l2 + m2)
    )
    kmin = max(0, l2 - l3 - m1, l1 - l3 + m2)
    kmax = min(l1 + l2 - l3, l1 - m1, l2 + m2)
    s = 0.0
    for k in range(kmin, kmax + 1):
        s += (-1) ** k / (
            factorial(k)
            * factorial(l1 + l2 - l3 - k)
            * factorial(l1 - m1 - k)
            * factorial(l2 + m2 - k)
            * factorial(l3 - l2 + m1 + k)
            * factorial(l3 - l1 - m2 + k)
        )
    return pre * s


def _u_c2r(l):
    U = np.zeros((2 * l + 1, 2 * l + 1), dtype=np.complex128)
    U[l, l] = 1.0
    for m in range(1, l + 1):
        U[l + m, l + m] = (-1) ** m / np.sqrt(2.0)
        U[l + m, l - m] = 1.0 / np.sqrt(2.0)
        U[l - m, l - m] = 1j / np.sqrt(2.0)
        U[l - m, l + m] = -1j * (-1) ** m / np.sqrt(2.0)
    return U


def _generate_cg_rsh(lmax):
    S = (lmax + 1) ** 2
    cg = np.zeros((S, S, S))
    for l1 in range(lmax + 1):
        for l2 in range(lmax + 1):
            for l3 in range(lmax + 1):
                if (l1 + l2 + l3) % 2 == 1 or not (abs(l1 - l2) <= l3 <= l1 + l2):
                    continue
                block = np.zeros(
                    (2 * l1 + 1, 2 * l2 + 1, 2 * l3 + 1), dtype=np.complex128
                )
                for m1 in range(-l1, l1 + 1):
                    for m2 in range(-l2, l2 + 1):
                        m3 = m1 + m2
                        if abs(m3) <= l3:
                            block[m1 + l1, m2 + l2, m3 + l3] = _cg_complex(
                                l1, m1, l2, m2, l3, m3
                            )
                rb = np.einsum(
                    "abc,ia,jb,kc->ijk",
                    block,
                    _u_c2r(l1),
                    _u_c2r(l2),
                    np.conj(_u_c2r(l3)),
                )
                cg[l1 * l1 : (l1 + 1) ** 2, l2 * l2 : (l2 + 1) ** 2, l3 * l3 : (l3 + 1) ** 2] = np.real(rb)
    cg[np.abs(cg) < 1e-12] = 0.0
    return cg


_CG = _generate_cg_rsh(LMAX)  # [s1, s2, o]
_LIDX = np.repeat(np.arange(LMAX + 1), 2 * np.arange(LMAX + 1) + 1)  # degree of s

# CGQ[q=(l1*9+s2), s1, o]: CG values masked to filter shell l1
_CGQ = np.zeros((NQ, SH, SH), np.float32)
for _l1 in range(3):
    for _s2 in range(SH):
        _m = (_LIDX == _l1).astype(np.float32)
        _CGQ[_l1 * SH + _s2] = _CG[:, _s2, :] * _m[:, None]


# ---------------------------------------------------------------------------
# Device program
# ---------------------------------------------------------------------------
_PROG_CACHE = {}


def _build_program(nb):
    from concourse import bacc, mybir
    from concourse.bass import IndirectOffsetOnAxis
    import concourse.tile as tile

    nc = bacc.Bacc("TRN2", target_bir_lowering=False, debug=False)
    xprep = nc.declare_dram_parameter(
        "xprep", [N_ATOMS + 1, SH * C], mybir.dt.bfloat16, False
    )
    idx = nc.declare_dram_parameter("idx", [BLK, nb], mybir.dt.int32, False)
    radT = nc.declare_dram_parameter("radT", [RAUG, nb * BLK], mybir.dt.bfloat16, False)
    wfa = nc.declare_dram_parameter("wfa", [RAUG, 3 * C], mybir.dt.bfloat16, False)
    stat = nc.declare_dram_parameter(
        "stat", [nb, BLK, NQ * 128], mybir.dt.bfloat16, False
    )
    yout = nc.declare_dram_parameter("yout", [128, nb * C], mybir.dt.float32, True)

    with tile.TileContext(nc) as tc:
        with (
            tc.tile_pool(name="const", bufs=1) as constp,
            tc.tile_pool(name="xjp", bufs=3) as xjp,
            tc.tile_pool(name="wbp", bufs=3) as wbp,
            tc.tile_pool(name="xjwp", bufs=3) as xjwp,
            tc.tile_pool(name="statp", bufs=3) as statp,
            tc.tile_pool(name="outp", bufs=1) as outp,
            tc.tile_pool(name="ps", bufs=2, space="PSUM") as ps,
            tc.tile_pool(name="psw", bufs=2, space="PSUM") as psw,
        ):
            idx_sb = constp.tile([BLK, nb], mybir.dt.int32)
            nc.sync.dma_start(out=idx_sb[:, :], in_=idx[:, :])
            radT_sb = constp.tile([RAUG, nb * BLK], mybir.dt.bfloat16)
            nc.sync.dma_start(out=radT_sb[:, :], in_=radT[:, :])
            wfa_sb = constp.tile([RAUG, 3 * C], mybir.dt.bfloat16)
            nc.sync.dma_start(out=wfa_sb[:, :], in_=wfa[:, :])
            y_sb = outp.tile([128, nb * C], mybir.dt.float32)

            for b in range(nb):
                xj = xjp.tile([BLK, SH * C], mybir.dt.bfloat16, tag="xj")
                nc.gpsimd.indirect_dma_start(
                    out=xj[:, :],
                    out_offset=None,
                    in_=xprep[:, :],
                    in_offset=IndirectOffsetOnAxis(ap=idx_sb[:, b : b + 1], axis=0),
                )
                statb = statp.tile([BLK, NQ * 128], mybir.dt.bfloat16, tag="stat")
                nc.sync.dma_start(out=statb[:, :], in_=stat[b, :, :])

                wps = psw.tile([128, 3 * C], mybir.dt.float32)
                nc.tensor.matmul(
                    out=wps[:, :],
                    lhsT=radT_sb[:, b * BLK : (b + 1) * BLK],
                    rhs=wfa_sb[:, :],
                    start=True,
                    stop=True,
                )
                wb = wbp.tile([128, 3 * C], mybir.dt.bfloat16, tag="wb")
                nc.any.tensor_copy(wb[:, :], wps[:, :])

                xjw = xjwp.tile([128, NQ * C], mybir.dt.bfloat16, tag="xjw")
                nc.vector.tensor_tensor(
                    out=xjw[:, :].rearrange("p (a b c) -> p a b c", a=3, b=SH, c=C),
                    in0=xj[:, :]
                    .rearrange("p (u b c) -> p u b c", u=1, b=SH, c=C)
                    .to_broadcast([128, 3, SH, C]),
                    in1=wb[:, :]
                    .rearrange("p (a u c) -> p a u c", a=3, u=1, c=C)
                    .to_broadcast([128, 3, SH, C]),
                    op=mybir.AluOpType.mult,
                )

                yps = ps.tile([128, C], mybir.dt.float32)
                for q in range(NQ):
                    nc.tensor.matmul(
                        out=yps[:, :],
                        lhsT=statb[:, q * 128 : (q + 1) * 128],
                        rhs=xjw[:, q * C : (q + 1) * C],
                        start=(q == 0),
                        stop=(q == NQ - 1),
                    )
                nc.any.tensor_copy(y_sb[:, b * C : (b + 1) * C], yps[:, :])

            nc.sync.dma_start(out=yout[:, :], in_=y_sb[:, :])
    nc.compile()
    return nc


def _get_program(nb):
    if nb not in _PROG_CACHE:
        _PROG_CACHE[nb] = _build_program(nb)
    return _PROG_CACHE[nb]


# ---------------------------------------------------------------------------
# Host-side prep / sharding
# ---------------------------------------------------------------------------
def _prep(x, radial_ij, dir_ij, cutoff_ij, Wf, bf, idx_i, idx_j):
    E = radial_ij.shape[0]
    A = x.shape[0]
    idx_i = np.asarray(idx_i).astype(np.int64)
    idx_j = np.asarray(idx_j).astype(np.int64)
    x = np.asarray(x, np.float32)
    dir_ij = np.asarray(dir_ij, np.float32)
    cutoff_ij = np.asarray(cutoff_ij, np.float32).reshape(E, 1)
    radial_ij = np.asarray(radial_ij, np.float32)
    Wf = np.asarray(Wf, np.float32)
    bf = np.asarray(bf, np.float32)

    radial_aug = np.concatenate([radial_ij * cutoff_ij, cutoff_ij], axis=1)  # [E,21]
    wfa = np.concatenate([Wf, bf[None, :]], axis=0).astype(BF16)  # [21, 384]

    # per-edge stationary values V[e, q, o]
    V = np.einsum("es,qso->eqo", dir_ij, _CGQ).astype(np.float32)  # [E, 27, 9]

    order = np.argsort(idx_i, kind="stable")
    ai_sorted = idx_i[order]

    counts = np.bincount(idx_i, minlength=A)
    cume = np.concatenate([[0], np.cumsum(counts)])  # edges before atom a
    # atom boundaries per core, balancing edge counts
    targets = [round(k * E / NCORES) for k in range(1, NCORES)]
    abounds = [0]
    for t in targets:
        a = int(np.searchsorted(cume, t, side="left"))
        a = max(min(a, A), abounds[-1])
        abounds.append(a)
    abounds.append(A)

    # block packing
    core_blocks = []  # per core: list of blocks; block = list of sorted-edge positions
    for k in range(NCORES):
        lo, hi = int(cume[abounds[k]]), int(cume[abounds[k + 1]])
        blocks = []
        cur = []
        base_atom = -1
        for t in range(lo, hi):
            a = int(ai_sorted[t])
            if cur and (len(cur) == BLK or a - base_atom >= RING):
                blocks.append(cur)
                cur = []
            if not cur:
                base_atom = a
            cur.append(t)
        if cur:
            blocks.append(cur)
        core_blocks.append(blocks)

    nb = max(len(b) for b in core_blocks)

    # flatten slots
    slot_core = []
    slot_b = []
    slot_s = []
    slot_e = []
    merge_plan = []  # (core, b, atoms_array)
    for k in range(NCORES):
        for b, blk in enumerate(core_blocks[k]):
            edges = order[np.asarray(blk, np.int64)]
            atoms = np.unique(idx_i[edges])
            merge_plan.append((k, b, atoms))
            for s, e in enumerate(edges):
                slot_core.append(k)
                slot_b.append(b)
                slot_s.append(s)
                slot_e.append(e)
    slot_core = np.asarray(slot_core, np.int64)
    slot_b = np.asarray(slot_b, np.int64)
    slot_s = np.asarray(slot_s, np.int64)
    slot_e = np.asarray(slot_e, np.int64)

    # per-core tensors
    idxT = np.full((NCORES, BLK, nb), A, np.int32)
    radT = np.zeros((NCORES, RAUG, nb * BLK), np.float32)
    stat = np.zeros((NCORES, nb, BLK, NQ, 128), np.float32)

    idxT[slot_core, slot_s, slot_b] = idx_j[slot_e].astype(np.int32)
    radT[slot_core, :, slot_b * BLK + slot_s] = radial_aug[slot_e]
    am = (idx_i[slot_e] % RING).astype(np.int64)  # [S]
    S = len(slot_e)
    cols = am[:, None, None] * SH + np.arange(SH)[None, None, :]  # [S, 1, 9]
    cols = np.broadcast_to(cols, (S, NQ, SH))
    stat[
        slot_core[:, None, None],
        slot_b[:, None, None],
        slot_s[:, None, None],
        np.arange(NQ)[None, :, None],
        cols,
    ] = V[slot_e]

    xprep = np.zeros((A + 1, SH * C), BF16)
    xprep[:A] = x.reshape(A, SH * C).astype(BF16)

    in_maps = []
    for k in range(NCORES):
        in_maps.append(
            {
                "xprep": xprep,
                "idx": idxT[k],
                "radT": radT[k].astype(BF16),
                "wfa": wfa,
                "stat": stat[k].reshape(nb, BLK, NQ * 128).astype(BF16),
            }
        )
    return nb, in_maps, merge_plan


def _merge(results, merge_plan, nb):
    y = np.zeros((N_ATOMS, SH, C), np.float32)
    for k, b, atoms in merge_plan:
        yo = results[k]["yout"]  # [128, nb*C]
        blkcols = yo[:, b * C : (b + 1) * C]
        for a in atoms:
            r = int(a % RING) * SH
            y[a] += blkcols[r : r + SH, :]
    return y


def run_kernel(inputs, trace=False):
    from concourse.bass_utils import run_bass_kernel_spmd

    nb, in_maps, merge_plan = _prep(
        inputs["x"],
        inputs["radial_ij"],
        inputs["dir_ij"],
        inputs["cutoff_ij"],
        inputs["Wf"],
        inputs["bf"],
        inputs["idx_i"],
        inputs["idx_j"],
    )
    nc = _get_program(nb)
    res = run_bass_kernel_spmd(nc, in_maps, list(range(NCORES)), trace=trace)
    y = _merge(res.results, merge_plan, nb)
    return y, res


def kernel(**inputs) -> np.ndarray:
    y, _ = run_kernel(inputs, trace=False)
    return y


# revision 9
# speedup vs baseline: 1.0726x; 1.0726x over previous
"""Trainium2 Bass kernel for nn_BaseSO3Convolution (SO(3) equivariant conv).

y[a,o,c] = sum_{e: idx_i[e]=a} sum_{paths (s1,s2,o)} CG[s1,s2,o] * dir[e,s1]
           * Wij[e, l(s1), c] * x[idx_j[e], s2, c]
with Wij = (radial @ Wf + bf) * cutoff.

Strategy (per core; 8 cores, edges sharded by receiving atom):
  - Edges sorted by idx_i, packed into 128-slot blocks whose atoms span < 14
    consecutive atoms.  For each block the entire CG contraction AND the
    within-block segment-sum are fused into 27 accumulating TensorE matmuls
    with contraction over the 128 edge slots:
        psum[(a mod 14)*9+o, c] += sum_e stat_q[e, (am,o)] * xjW[e, q, c]
    where q=(l1,s2) and stat_q[e, col] = CG-coef * dir placed at the edge's
    receiving-atom ring position (host-built, bf16).
  - xjW[e,(l1,s2),c] = Wij[e,l1,c] * xj[e,s2,c] is one broadcast DVE multiply.
  - xj gathered by indirect DMA from x (bf16) using idx_j.
  - Wij computed on-device by a small matmul (radialAug^T stationary).
  - Per-block partial sums land in a [128, NB*128] output; the host merges
    block partials into y (a few thousand 9x128 adds).
"""

import sys

sys.path.insert(0, "/opt/trn_rl_repo")

import numpy as np
import ml_dtypes
from math import factorial, sqrt

BF16 = ml_dtypes.bfloat16

LMAX = 2
SH = 9
N_ATOMS = 1000
N_EDGES = 10000
C = 128
NR = 20
RAUG = NR + 1  # radial basis augmented with cutoff column (bias folding)
NCORES = 8
BLK = 128  # edge slots per block
RING = 14  # atom ring size (mod-14 placement), 14*9=126 <= 128 psum partitions
NQ = 27  # (l1, s2) combos


# ---------------------------------------------------------------------------
# Clebsch-Gordan (real spherical harmonics) — self-contained copy
# ---------------------------------------------------------------------------
def _cg_complex(l1, m1, l2, m2, l3, m3):
    if m3 != m1 + m2 or not (abs(l1 - l2) <= l3 <= l1 + l2):
        return 0.0
    pre = sqrt(
        (2 * l3 + 1)
        * factorial(l3 + l1 - l2)
        * factorial(l3 - l1 + l2)
        * factorial(l1 + l2 - l3)
        / factorial(l1 + l2 + l3 + 1)
    )
    pre *= sqrt(
        factorial(l3 + m3)
        * factorial(l3 - m3)
        * factorial(l1 - m1)
        * factorial(l1 + m1)
        * factorial(l2 - m2)
        * factorial(l2 + m2)
    )
    kmin = max(0, l2 - l3 - m1, l1 - l3 + m2)
    kmax = min(l1 + l2 - l3, l1 - m1, l2 + m2)
    s = 0.0
    for k in range(kmin, kmax + 1):
        s += (-1) ** k / (
            factorial(k)
            * factorial(l1 + l2 - l3 - k)
            * factorial(l1 - m1 - k)
            * factorial(l2 + m2 - k)
            * factorial(l3 - l2 + m1 + k)
            * factorial(l3 - l1 - m2 + k)
        )
    return pre * s


def _u_c2r(l):
    U = np.zeros((2 * l + 1, 2 * l + 1), dtype=np.complex128)
    U[l, l] = 1.0
    for m in range(1, l + 1):
        U[l + m, l + m] = (-1) ** m / np.sqrt(2.0)
        U[l + m, l - m] = 1.0 / np.sqrt(2.0)
        U[l - m, l - m] = 1j / np.sqrt(2.0)
        U[l - m, l + m] = -1j * (-1) ** m / np.sqrt(2.0)
    return U


def _generate_cg_rsh(lmax):
    S = (lmax + 1) ** 2
    cg = np.zeros((S, S, S))
    for l1 in range(lmax + 1):
        for l2 in range(lmax + 1):
            for l3 in range(lmax + 1):
                if (l1 + l2 + l3) % 2 == 1 or not (abs(l1 - l2) <= l3 <= l1 + l2):
                    continue
                block = np.zeros(
                    (2 * l1 + 1, 2 * l2 + 1, 2 * l3 + 1), dtype=np.complex128
                )
                for m1 in range(-l1, l1 + 1):
                    for m2 in range(-l2, l2 + 1):
                        m3 = m1 + m2
                        if abs(m3) <= l3:
                            block[m1 + l1, m2 + l2, m3 + l3] = _cg_complex(
                                l1, m1, l2, m2, l3, m3
                            )
                rb = np.einsum(
                    "abc,ia,jb,kc->ijk",
                    block,
                    _u_c2r(l1),
                    _u_c2r(l2),
                    np.conj(_u_c2r(l3)),
                )
                cg[l1 * l1 : (l1 + 1) ** 2, l2 * l2 : (l2 + 1) ** 2, l3 * l3 : (l3 + 1) ** 2] = np.real(rb)
    cg[np.abs(cg) < 1e-12] = 0.0
    return cg


_CG = _generate_cg_rsh(LMAX)  # [s1, s2, o]
_LIDX = np.repeat(np.arange(LMAX + 1), 2 * np.arange(LMAX + 1) + 1)  # degree of s

# CGQ[q=(l1*9+s2), s1, o]: CG values masked to filter shell l1
_CGQ = np.zeros((NQ, SH, SH), np.float32)
for _l1 in range(3):
    for _s2 in range(SH):
        _m = (_LIDX == _l1).astype(np.float32)
        _CGQ[_l1 * SH + _s2] = _CG[:, _s2, :] * _m[:, None]

# --- stationary compression structure --------------------------------------
# l1=0: CG[0,s2,o] = delta(s2,o) exactly -> one rank-1 stationary [e, am]
#       (dir[e,0]) with moving xjW[:, l1=0 slice] and output [am, (s2,c)].
# l1>=1: per-q stationary covering only the o-span it needs, with o's
#       relabeled by _POS to cluster co-occurring o's, and the PSUM base
#       partition 32-aligned per the TensorE tile-position rules.
_POS = np.array([3, 0, 1, 2, 4, 5, 6, 7, 8])  # o -> ring position
_IPOS = np.argsort(_POS)  # position -> o

_RUNS = []  # (q, base, m, col_off) for q in 9..26
for _q in range(SH, NQ):
    _os = np.where(np.abs(_CGQ[_q]).sum(axis=0) > 0)[0]
    _hi = (int(_POS[_os].max()) + 1) * RING
    _RUNS.append((_q, 0, _hi, 0))
# The first accumulating matmul must cover the WHOLE psum tile: PSUM's
# start=True clears has_written only for elements it writes; rows first
# touched by a later start=False matmul would accumulate stale data.
_RUNS.sort(key=lambda r: -r[2])
_RUNS[0] = (_RUNS[0][0], 0, 128, 0)
_off = RING  # cols [0, RING) hold the l1=0 stationary (dir[e,0] at col am)
for _i, (_q, _b, _m, _) in enumerate(_RUNS):
    _RUNS[_i] = (_q, _b, _m, _off)
    _off += _m
TOTC = _off  # total stationary columns per block


# ---------------------------------------------------------------------------
# Device program
# ---------------------------------------------------------------------------
_PROG_CACHE = {}


def _build_program(nb):
    from concourse import bacc, mybir
    from concourse.bass import IndirectOffsetOnAxis
    import concourse.tile as tile

    nc = bacc.Bacc("TRN2", target_bir_lowering=False, debug=False)
    xprep = nc.declare_dram_parameter(
        "xprep", [N_ATOMS + 1, SH * C], mybir.dt.bfloat16, False
    )
    idx = nc.declare_dram_parameter("idx", [BLK, nb], mybir.dt.int32, False)
    radT = nc.declare_dram_parameter("radT", [RAUG, nb * BLK], mybir.dt.bfloat16, False)
    wfa = nc.declare_dram_parameter("wfa", [RAUG, 3 * C], mybir.dt.bfloat16, False)
    stat = nc.declare_dram_parameter("stat", [nb, BLK, TOTC], mybir.dt.bfloat16, False)
    yout = nc.declare_dram_parameter("yout", [128, nb * C], mybir.dt.float32, True)
    yout0 = nc.declare_dram_parameter(
        "yout0", [RING, nb * SH * C], mybir.dt.float32, True
    )

    with tile.TileContext(nc) as tc:
        with (
            tc.tile_pool(name="const", bufs=1) as constp,
            tc.tile_pool(name="xjp", bufs=3) as xjp,
            tc.tile_pool(name="wbp", bufs=3) as wbp,
            tc.tile_pool(name="xjwp", bufs=3) as xjwp,
            tc.tile_pool(name="statp", bufs=3) as statp,
            tc.tile_pool(name="outp", bufs=1) as outp,
            tc.tile_pool(name="ps", bufs=2, space="PSUM") as ps,
            tc.tile_pool(name="ps0", bufs=1, space="PSUM") as ps0,
            tc.tile_pool(name="psw", bufs=2, space="PSUM") as psw,
        ):
            idx_sb = constp.tile([BLK, nb], mybir.dt.int32)
            nc.sync.dma_start(out=idx_sb[:, :], in_=idx[:, :])
            radT_sb = constp.tile([RAUG, nb * BLK], mybir.dt.bfloat16)
            nc.sync.dma_start(out=radT_sb[:, :], in_=radT[:, :])
            wfa_sb = constp.tile([RAUG, 3 * C], mybir.dt.bfloat16)
            nc.sync.dma_start(out=wfa_sb[:, :], in_=wfa[:, :])
            y_sb = outp.tile([128, nb * C], mybir.dt.float32)
            y0_sb = outp.tile([RING, nb * SH * C], mybir.dt.float32)

            for b in range(nb):
                xj = xjp.tile([BLK, SH * C], mybir.dt.bfloat16, tag="xj")
                nc.gpsimd.indirect_dma_start(
                    out=xj[:, :],
                    out_offset=None,
                    in_=xprep[:, :],
                    in_offset=IndirectOffsetOnAxis(ap=idx_sb[:, b : b + 1], axis=0),
                )
                statb = statp.tile([BLK, TOTC], mybir.dt.bfloat16, tag="stat")
                nc.sync.dma_start(out=statb[:, :], in_=stat[b, :, :])

                wps = psw.tile([128, 3 * C], mybir.dt.float32)
                nc.tensor.matmul(
                    out=wps[:, :],
                    lhsT=radT_sb[:, b * BLK : (b + 1) * BLK],
                    rhs=wfa_sb[:, :],
                    start=True,
                    stop=True,
                )
                wb = wbp.tile([128, 3 * C], mybir.dt.bfloat16, tag="wb")
                nc.scalar.copy(wb[:, :], wps[:, :])

                xjw = xjwp.tile([128, NQ * C], mybir.dt.bfloat16, tag="xjw")
                nc.vector.tensor_tensor(
                    out=xjw[:, :].rearrange("p (a b c) -> p a b c", a=3, b=SH, c=C),
                    in0=xj[:, :]
                    .rearrange("p (u b c) -> p u b c", u=1, b=SH, c=C)
                    .to_broadcast([128, 3, SH, C]),
                    in1=wb[:, :]
                    .rearrange("p (a u c) -> p a u c", a=3, u=1, c=C)
                    .to_broadcast([128, 3, SH, C]),
                    op=mybir.AluOpType.mult,
                )

                # l1=0 rank-1 path: out0[am, (s2,c)] += sum_e dir0[e]*xjW0[e,(s2,c)]
                yp0 = ps0.tile([RING, SH * C], mybir.dt.float32)
                for n0, n1 in ((0, 512), (512, 1024), (1024, 1152)):
                    # each chunk stays within one 2KB psum bank
                    nc.tensor.matmul(
                        out=yp0[:, n0:n1],
                        lhsT=statb[:, 0:RING],
                        rhs=xjw[:, n0:n1],
                        start=True,
                        stop=True,
                    )
                nc.scalar.copy(
                    y0_sb[:, b * SH * C : (b + 1) * SH * C], yp0[:, :]
                )

                # l1>=1: span-compressed runs into one ring psum tile
                yps = ps.tile([128, C], mybir.dt.float32)
                nrun = len(_RUNS)
                for i, (q, base, m, off) in enumerate(_RUNS):
                    nc.tensor.matmul(
                        out=yps[base : base + m, :],
                        lhsT=statb[:, off : off + m],
                        rhs=xjw[:, q * C : (q + 1) * C],
                        start=(i == 0),
                        stop=(i == nrun - 1),
                        skip_group_check=True,
                    )
                nc.vector.tensor_copy(y_sb[:, b * C : (b + 1) * C], yps[:, :])

            nc.sync.dma_start(out=yout[:, :], in_=y_sb[:, :])
            nc.sync.dma_start(out=yout0[:, :], in_=y0_sb[:, :])
    nc.compile()
    return nc


def _get_program(nb):
    if nb not in _PROG_CACHE:
        _PROG_CACHE[nb] = _build_program(nb)
    return _PROG_CACHE[nb]


# ---------------------------------------------------------------------------
# Host-side prep / sharding
# ---------------------------------------------------------------------------
def _prep(x, radial_ij, dir_ij, cutoff_ij, Wf, bf, idx_i, idx_j):
    E = radial_ij.shape[0]
    A = x.shape[0]
    idx_i = np.asarray(idx_i).astype(np.int64)
    idx_j = np.asarray(idx_j).astype(np.int64)
    x = np.asarray(x, np.float32)
    dir_ij = np.asarray(dir_ij, np.float32)
    cutoff_ij = np.asarray(cutoff_ij, np.float32).reshape(E, 1)
    radial_ij = np.asarray(radial_ij, np.float32)
    Wf = np.asarray(Wf, np.float32)
    bf = np.asarray(bf, np.float32)

    radial_aug = np.concatenate([radial_ij * cutoff_ij, cutoff_ij], axis=1)  # [E,21]
    wfa = np.concatenate([Wf, bf[None, :]], axis=0).astype(BF16)  # [21, 384]

    # per-edge stationary values V[e, q, o]
    V = np.einsum("es,qso->eqo", dir_ij, _CGQ).astype(np.float32)  # [E, 27, 9]

    order = np.argsort(idx_i, kind="stable")
    ai_sorted = idx_i[order]

    counts = np.bincount(idx_i, minlength=A)
    cume = np.concatenate([[0], np.cumsum(counts)])  # edges before atom a
    # atom boundaries per core, balancing edge counts
    targets = [round(k * E / NCORES) for k in range(1, NCORES)]
    abounds = [0]
    for t in targets:
        a = int(np.searchsorted(cume, t, side="left"))
        a = max(min(a, A), abounds[-1])
        abounds.append(a)
    abounds.append(A)

    # block packing
    core_blocks = []  # per core: list of blocks; block = list of sorted-edge positions
    for k in range(NCORES):
        lo, hi = int(cume[abounds[k]]), int(cume[abounds[k + 1]])
        blocks = []
        cur = []
        base_atom = -1
        for t in range(lo, hi):
            a = int(ai_sorted[t])
            if cur and (len(cur) == BLK or a - base_atom >= RING):
                blocks.append(cur)
                cur = []
            if not cur:
                base_atom = a
            cur.append(t)
        if cur:
            blocks.append(cur)
        core_blocks.append(blocks)

    nb = max(len(b) for b in core_blocks)

    # flatten slots
    slot_core = []
    slot_b = []
    slot_s = []
    slot_e = []
    merge_plan = []  # (core, b, atoms_array)
    for k in range(NCORES):
        for b, blk in enumerate(core_blocks[k]):
            edges = order[np.asarray(blk, np.int64)]
            atoms = np.unique(idx_i[edges])
            merge_plan.append((k, b, atoms))
            for s, e in enumerate(edges):
                slot_core.append(k)
                slot_b.append(b)
                slot_s.append(s)
                slot_e.append(e)
    slot_core = np.asarray(slot_core, np.int64)
    slot_b = np.asarray(slot_b, np.int64)
    slot_s = np.asarray(slot_s, np.int64)
    slot_e = np.asarray(slot_e, np.int64)

    # per-core tensors
    idxT = np.full((NCORES, BLK, nb), A, np.int32)
    radT = np.zeros((NCORES, RAUG, nb * BLK), np.float32)
    stat = np.zeros((NCORES, nb, BLK, TOTC), np.float32)

    idxT[slot_core, slot_s, slot_b] = idx_j[slot_e].astype(np.int32)
    radT[slot_core, :, slot_b * BLK + slot_s] = radial_aug[slot_e]
    am = (idx_i[slot_e] % RING).astype(np.int64)  # [S]
    # l1=0 stationary: dir[e,0] at column am
    stat[slot_core, slot_b, slot_s, am] = dir_ij[slot_e, 0]
    # l1>=1 runs
    for q, base, m, off in _RUNS:
        osel = np.where(np.abs(_CGQ[q]).sum(axis=0) > 0)[0]
        cols = off + _POS[osel][None, :] * RING - base + am[:, None]  # [S, |osel|]
        stat[
            slot_core[:, None],
            slot_b[:, None],
            slot_s[:, None],
            cols,
        ] = V[slot_e][:, q, osel]

    xprep = np.zeros((A + 1, SH * C), BF16)
    xprep[:A] = x.reshape(A, SH * C).astype(BF16)

    in_maps = []
    for k in range(NCORES):
        in_maps.append(
            {
                "xprep": xprep,
                "idx": idxT[k],
                "radT": radT[k].astype(BF16),
                "wfa": wfa,
                "stat": stat[k].astype(BF16),
            }
        )
    return nb, in_maps, merge_plan


def _merge(results, merge_plan, nb):
    y = np.zeros((N_ATOMS, SH, C), np.float32)
    rows = _POS * RING  # o -> psum ring row block start (adds am)
    for k, b, atoms in merge_plan:
        yo = results[k]["yout"]  # [128, nb*C], partition = POS[o]*RING + am
        yo0 = results[k]["yout0"]  # [RING, nb*SH*C], l1=0 partial [am, (s2, c)]
        blkcols = yo[:, b * C : (b + 1) * C]
        blk0 = yo0[:, b * SH * C : (b + 1) * SH * C]
        for a in atoms:
            am = int(a % RING)
            y[a] += blkcols[rows + am, :]
            y[a] += blk0[am].reshape(SH, C)
    return y


def run_kernel(inputs, trace=False):
    from concourse.bass_utils import run_bass_kernel_spmd

    nb, in_maps, merge_plan = _prep(
        inputs["x"],
        inputs["radial_ij"],
        inputs["dir_ij"],
        inputs["cutoff_ij"],
        inputs["Wf"],
        inputs["bf"],
        inputs["idx_i"],
        inputs["idx_j"],
    )
    nc = _get_program(nb)
    res = run_bass_kernel_spmd(nc, in_maps, list(range(NCORES)), trace=trace)
    y = _merge(res.results, merge_plan, nb)
    return y, res


def kernel(**inputs) -> np.ndarray:
    y, _ = run_kernel(inputs, trace=False)
    return y


# revision 19
# speedup vs baseline: 1.1662x; 1.0873x over previous
"""Trainium2 Bass kernel for nn_BaseSO3Convolution (SO(3) equivariant conv).

y[a,o,c] = sum_{e: idx_i[e]=a} sum_{paths (s1,s2,o)} CG[s1,s2,o] * dir[e,s1]
           * Wij[e, l(s1), c] * x[idx_j[e], s2, c]
with Wij = (radial @ Wf + bf) * cutoff.

Strategy (per core; 8 cores, edges sharded by receiving atom):
  - Edges sorted by idx_i, packed into 128-slot blocks whose atoms span < 14
    consecutive atoms.  For each block the entire CG contraction AND the
    within-block segment-sum are fused into 27 accumulating TensorE matmuls
    with contraction over the 128 edge slots:
        psum[(a mod 14)*9+o, c] += sum_e stat_q[e, (am,o)] * xjW[e, q, c]
    where q=(l1,s2) and stat_q[e, col] = CG-coef * dir placed at the edge's
    receiving-atom ring position (host-built, bf16).
  - xjW[e,(l1,s2),c] = Wij[e,l1,c] * xj[e,s2,c] is one broadcast DVE multiply.
  - xj gathered by indirect DMA from x (bf16) using idx_j.
  - Wij computed on-device by a small matmul (radialAug^T stationary).
  - Per-block partial sums land in a [128, NB*128] output; the host merges
    block partials into y (a few thousand 9x128 adds).
"""

import sys

sys.path.insert(0, "/opt/trn_rl_repo")

import numpy as np
import ml_dtypes
from math import factorial, sqrt

BF16 = ml_dtypes.bfloat16

LMAX = 2
SH = 9
N_ATOMS = 1000
N_EDGES = 10000
C = 128
NR = 20
RAUG = NR + 1  # radial basis augmented with cutoff column (bias folding)
NCORES = 8
BLK = 128  # edge slots per block
RING = 14  # atom ring size (mod-14 placement), 14*9=126 <= 128 psum partitions
NQ = 27  # (l1, s2) combos


# ---------------------------------------------------------------------------
# Clebsch-Gordan (real spherical harmonics) — self-contained copy
# ---------------------------------------------------------------------------
def _cg_complex(l1, m1, l2, m2, l3, m3):
    if m3 != m1 + m2 or not (abs(l1 - l2) <= l3 <= l1 + l2):
        return 0.0
    pre = sqrt(
        (2 * l3 + 1)
        * factorial(l3 + l1 - l2)
        * factorial(l3 - l1 + l2)
        * factorial(l1 + l2 - l3)
        / factorial(l1 + l2 + l3 + 1)
    )
    pre *= sqrt(
        factorial(l3 + m3)
        * factorial(l3 - m3)
        * factorial(l1 - m1)
        * factorial(l1 + m1)
        * factorial(l2 - m2)
        * factorial(l2 + m2)
    )
    kmin = max(0, l2 - l3 - m1, l1 - l3 + m2)
    kmax = min(l1 + l2 - l3, l1 - m1, l2 + m2)
    s = 0.0
    for k in range(kmin, kmax + 1):
        s += (-1) ** k / (
            factorial(k)
            * factorial(l1 + l2 - l3 - k)
            * factorial(l1 - m1 - k)
            * factorial(l2 + m2 - k)
            * factorial(l3 - l2 + m1 + k)
            * factorial(l3 - l1 - m2 + k)
        )
    return pre * s


def _u_c2r(l):
    U = np.zeros((2 * l + 1, 2 * l + 1), dtype=np.complex128)
    U[l, l] = 1.0
    for m in range(1, l + 1):
        U[l + m, l + m] = (-1) ** m / np.sqrt(2.0)
        U[l + m, l - m] = 1.0 / np.sqrt(2.0)
        U[l - m, l - m] = 1j / np.sqrt(2.0)
        U[l - m, l + m] = -1j * (-1) ** m / np.sqrt(2.0)
    return U


def _generate_cg_rsh(lmax):
    S = (lmax + 1) ** 2
    cg = np.zeros((S, S, S))
    for l1 in range(lmax + 1):
        for l2 in range(lmax + 1):
            for l3 in range(lmax + 1):
                if (l1 + l2 + l3) % 2 == 1 or not (abs(l1 - l2) <= l3 <= l1 + l2):
                    continue
                block = np.zeros(
                    (2 * l1 + 1, 2 * l2 + 1, 2 * l3 + 1), dtype=np.complex128
                )
                for m1 in range(-l1, l1 + 1):
                    for m2 in range(-l2, l2 + 1):
                        m3 = m1 + m2
                        if abs(m3) <= l3:
                            block[m1 + l1, m2 + l2, m3 + l3] = _cg_complex(
                                l1, m1, l2, m2, l3, m3
                            )
                rb = np.einsum(
                    "abc,ia,jb,kc->ijk",
                    block,
                    _u_c2r(l1),
                    _u_c2r(l2),
                    np.conj(_u_c2r(l3)),
                )
                cg[l1 * l1 : (l1 + 1) ** 2, l2 * l2 : (l2 + 1) ** 2, l3 * l3 : (l3 + 1) ** 2] = np.real(rb)
    cg[np.abs(cg) < 1e-12] = 0.0
    return cg


_CG = _generate_cg_rsh(LMAX)  # [s1, s2, o]
_LIDX = np.repeat(np.arange(LMAX + 1), 2 * np.arange(LMAX + 1) + 1)  # degree of s

# CGQ[q=(l1*9+s2), s1, o]: CG values masked to filter shell l1
_CGQ = np.zeros((NQ, SH, SH), np.float32)
for _l1 in range(3):
    for _s2 in range(SH):
        _m = (_LIDX == _l1).astype(np.float32)
        _CGQ[_l1 * SH + _s2] = _CG[:, _s2, :] * _m[:, None]

# --- stationary compression structure --------------------------------------
# l1=0: CG[0,s2,o] = delta(s2,o) exactly -> one rank-1 stationary [e, am]
#       (dir[e,0]) with moving xjW[:, l1=0 slice] and output [am, (s2,c)].
# l1>=1: per-q stationary covering only the o-span it needs, with o's
#       relabeled by _POS to cluster co-occurring o's, and the PSUM base
#       partition 32-aligned per the TensorE tile-position rules.
_POS = np.array([3, 0, 1, 2, 4, 5, 6, 7, 8])  # o -> ring position
_IPOS = np.argsort(_POS)  # position -> o

_RUNS = []  # (q, base, m, col_off) for q in 9..26
for _q in range(SH, NQ):
    _os = np.where(np.abs(_CGQ[_q]).sum(axis=0) > 0)[0]
    _hi = (int(_POS[_os].max()) + 1) * RING
    _RUNS.append((_q, 0, _hi, 0))
# The first accumulating matmul must cover the WHOLE psum tile: PSUM's
# start=True clears has_written only for elements it writes; rows first
# touched by a later start=False matmul would accumulate stale data.
_RUNS.sort(key=lambda r: -r[2])
_RUNS[0] = (_RUNS[0][0], 0, 128, 0)
_off = RING  # cols [0, RING) hold the l1=0 stationary (dir[e,0] at col am)
for _i, (_q, _b, _m, _) in enumerate(_RUNS):
    _RUNS[_i] = (_q, _b, _m, _off)
    _off += _m
TOTC = _off  # total stationary columns per block


# ---------------------------------------------------------------------------
# Device program
# ---------------------------------------------------------------------------
_PROG_CACHE = {}


def _build_program(nb):
    from concourse import bacc, mybir
    from concourse.bass import IndirectOffsetOnAxis
    import concourse.tile as tile

    nc = bacc.Bacc("TRN2", target_bir_lowering=False, debug=False)
    xprep = nc.declare_dram_parameter(
        "xprep", [N_ATOMS + 1, SH * C], mybir.dt.bfloat16, False
    )
    idx = nc.declare_dram_parameter("idx", [BLK, nb], mybir.dt.int32, False)
    radT = nc.declare_dram_parameter("radT", [RAUG, nb * BLK], mybir.dt.bfloat16, False)
    wfa = nc.declare_dram_parameter("wfa", [RAUG, 3 * C], mybir.dt.bfloat16, False)
    stat = nc.declare_dram_parameter("stat", [nb, BLK, TOTC], mybir.dt.bfloat16, False)
    yout = nc.declare_dram_parameter("yout", [128, nb * C], mybir.dt.float32, True)
    yout0 = nc.declare_dram_parameter(
        "yout0", [RING, nb * SH * C], mybir.dt.float32, True
    )

    with tile.TileContext(nc) as tc:
        with (
            tc.tile_pool(name="const", bufs=1) as constp,
            tc.tile_pool(name="xjp", bufs=nb) as xjp,
            tc.tile_pool(name="wbp", bufs=3) as wbp,
            tc.tile_pool(name="xjwp", bufs=3) as xjwp,
            tc.tile_pool(name="statp", bufs=nb) as statp,
            tc.tile_pool(name="outp", bufs=1) as outp,
            tc.tile_pool(name="ps", bufs=2, space="PSUM") as ps,
            tc.tile_pool(name="ps0", bufs=1, space="PSUM") as ps0,
            tc.tile_pool(name="psw", bufs=2, space="PSUM") as psw,
        ):
            idx_sb = constp.tile([BLK, nb], mybir.dt.int32)
            nc.sync.dma_start(out=idx_sb[:, :], in_=idx[:, :])
            radT_sb = constp.tile([RAUG, nb * BLK], mybir.dt.bfloat16)
            nc.sync.dma_start(out=radT_sb[:, :], in_=radT[:, :])
            wfa_sb = constp.tile([RAUG, 3 * C], mybir.dt.bfloat16)
            nc.sync.dma_start(out=wfa_sb[:, :], in_=wfa[:, :])
            y_sb = outp.tile([128, nb * C], mybir.dt.float32)
            y0_sb = outp.tile([RING, nb * SH * C], mybir.dt.float32)

            for b in range(nb):
                xj = xjp.tile([BLK, SH * C], mybir.dt.bfloat16, tag="xj")
                nc.gpsimd.indirect_dma_start(
                    out=xj[:, :],
                    out_offset=None,
                    in_=xprep[:, :],
                    in_offset=IndirectOffsetOnAxis(ap=idx_sb[:, b : b + 1], axis=0),
                )
                statb = statp.tile([BLK, TOTC], mybir.dt.bfloat16, tag="stat")
                nc.sync.dma_start(out=statb[:, :], in_=stat[b, :, :])

                wps = psw.tile([128, 3 * C], mybir.dt.float32)
                nc.tensor.matmul(
                    out=wps[:, :],
                    lhsT=radT_sb[:, b * BLK : (b + 1) * BLK],
                    rhs=wfa_sb[:, :],
                    start=True,
                    stop=True,
                )
                wb = wbp.tile([128, 3 * C], mybir.dt.bfloat16, tag="wb")
                nc.scalar.copy(wb[:, :], wps[:, :])

                xjw = xjwp.tile([128, NQ * C], mybir.dt.bfloat16, tag="xjw")
                nc.vector.tensor_tensor(
                    out=xjw[:, :].rearrange("p (a b c) -> p a b c", a=3, b=SH, c=C),
                    in0=xj[:, :]
                    .rearrange("p (u b c) -> p u b c", u=1, b=SH, c=C)
                    .to_broadcast([128, 3, SH, C]),
                    in1=wb[:, :]
                    .rearrange("p (a u c) -> p a u c", a=3, u=1, c=C)
                    .to_broadcast([128, 3, SH, C]),
                    op=mybir.AluOpType.mult,
                )

                # l1=0 rank-1 path: out0[am, (s2,c)] += sum_e dir0[e]*xjW0[e,(s2,c)]
                yp0 = ps0.tile([RING, SH * C], mybir.dt.float32)
                for n0, n1 in ((0, 512), (512, 1024), (1024, 1152)):
                    # each chunk stays within one 2KB psum bank
                    nc.tensor.matmul(
                        out=yp0[:, n0:n1],
                        lhsT=statb[:, 0:RING],
                        rhs=xjw[:, n0:n1],
                        start=True,
                        stop=True,
                    )
                nc.scalar.copy(
                    y0_sb[:, b * SH * C : (b + 1) * SH * C], yp0[:, :]
                )

                # l1>=1: span-compressed runs into one ring psum tile
                yps = ps.tile([128, C], mybir.dt.float32)
                nrun = len(_RUNS)
                for i, (q, base, m, off) in enumerate(_RUNS):
                    nc.tensor.matmul(
                        out=yps[base : base + m, :],
                        lhsT=statb[:, off : off + m],
                        rhs=xjw[:, q * C : (q + 1) * C],
                        start=(i == 0),
                        stop=(i == nrun - 1),
                        skip_group_check=True,
                    )
                nc.vector.tensor_copy(y_sb[:, b * C : (b + 1) * C], yps[:, :])

            nc.sync.dma_start(out=yout[:, :], in_=y_sb[:, :])
            nc.sync.dma_start(out=yout0[:, :], in_=y0_sb[:, :])
    nc.compile()
    return nc


def _get_program(nb):
    if nb not in _PROG_CACHE:
        _PROG_CACHE[nb] = _build_program(nb)
    return _PROG_CACHE[nb]


# ---------------------------------------------------------------------------
# Host-side prep / sharding
# ---------------------------------------------------------------------------
def _prep(x, radial_ij, dir_ij, cutoff_ij, Wf, bf, idx_i, idx_j):
    E = radial_ij.shape[0]
    A = x.shape[0]
    idx_i = np.asarray(idx_i).astype(np.int64)
    idx_j = np.asarray(idx_j).astype(np.int64)
    x = np.asarray(x, np.float32)
    dir_ij = np.asarray(dir_ij, np.float32)
    cutoff_ij = np.asarray(cutoff_ij, np.float32).reshape(E, 1)
    radial_ij = np.asarray(radial_ij, np.float32)
    Wf = np.asarray(Wf, np.float32)
    bf = np.asarray(bf, np.float32)

    radial_aug = np.concatenate([radial_ij * cutoff_ij, cutoff_ij], axis=1)  # [E,21]
    wfa = np.concatenate([Wf, bf[None, :]], axis=0).astype(BF16)  # [21, 384]

    # per-edge stationary values V[e, q, o]
    V = np.einsum("es,qso->eqo", dir_ij, _CGQ).astype(np.float32)  # [E, 27, 9]

    order = np.argsort(idx_i, kind="stable")
    ai_sorted = idx_i[order]

    counts = np.bincount(idx_i, minlength=A)
    cume = np.concatenate([[0], np.cumsum(counts)])  # edges before atom a
    # atom boundaries per core, balancing edge counts
    targets = [round(k * E / NCORES) for k in range(1, NCORES)]
    abounds = [0]
    for t in targets:
        a = int(np.searchsorted(cume, t, side="left"))
        a = max(min(a, A), abounds[-1])
        abounds.append(a)
    abounds.append(A)

    # block packing
    core_blocks = []  # per core: list of blocks; block = list of sorted-edge positions
    for k in range(NCORES):
        lo, hi = int(cume[abounds[k]]), int(cume[abounds[k + 1]])
        blocks = []
        cur = []
        base_atom = -1
        for t in range(lo, hi):
            a = int(ai_sorted[t])
            if cur and (len(cur) == BLK or a - base_atom >= RING):
                blocks.append(cur)
                cur = []
            if not cur:
                base_atom = a
            cur.append(t)
        if cur:
            blocks.append(cur)
        core_blocks.append(blocks)

    nb = max(len(b) for b in core_blocks)

    # flatten slots
    slot_core = []
    slot_b = []
    slot_s = []
    slot_e = []
    merge_plan = []  # (core, b, atoms_array)
    for k in range(NCORES):
        for b, blk in enumerate(core_blocks[k]):
            edges = order[np.asarray(blk, np.int64)]
            atoms = np.unique(idx_i[edges])
            merge_plan.append((k, b, atoms))
            for s, e in enumerate(edges):
                slot_core.append(k)
                slot_b.append(b)
                slot_s.append(s)
                slot_e.append(e)
    slot_core = np.asarray(slot_core, np.int64)
    slot_b = np.asarray(slot_b, np.int64)
    slot_s = np.asarray(slot_s, np.int64)
    slot_e = np.asarray(slot_e, np.int64)

    # per-core tensors
    idxT = np.full((NCORES, BLK, nb), A, np.int32)
    radT = np.zeros((NCORES, RAUG, nb * BLK), np.float32)
    stat = np.zeros((NCORES, nb, BLK, TOTC), np.float32)

    idxT[slot_core, slot_s, slot_b] = idx_j[slot_e].astype(np.int32)
    radT[slot_core, :, slot_b * BLK + slot_s] = radial_aug[slot_e]
    am = (idx_i[slot_e] % RING).astype(np.int64)  # [S]
    # l1=0 stationary: dir[e,0] at column am
    stat[slot_core, slot_b, slot_s, am] = dir_ij[slot_e, 0]
    # l1>=1 runs
    for q, base, m, off in _RUNS:
        osel = np.where(np.abs(_CGQ[q]).sum(axis=0) > 0)[0]
        cols = off + _POS[osel][None, :] * RING - base + am[:, None]  # [S, |osel|]
        stat[
            slot_core[:, None],
            slot_b[:, None],
            slot_s[:, None],
            cols,
        ] = V[slot_e][:, q, osel]

    xprep = np.zeros((A + 1, SH * C), BF16)
    xprep[:A] = x.reshape(A, SH * C).astype(BF16)

    in_maps = []
    for k in range(NCORES):
        in_maps.append(
            {
                "xprep": xprep,
                "idx": idxT[k],
                "radT": radT[k].astype(BF16),
                "wfa": wfa,
                "stat": stat[k].astype(BF16),
            }
        )
    return nb, in_maps, merge_plan


def _merge(results, merge_plan, nb):
    y = np.zeros((N_ATOMS, SH, C), np.float32)
    rows = _POS * RING  # o -> psum ring row block start (adds am)
    for k, b, atoms in merge_plan:
        yo = results[k]["yout"]  # [128, nb*C], partition = POS[o]*RING + am
        yo0 = results[k]["yout0"]  # [RING, nb*SH*C], l1=0 partial [am, (s2, c)]
        blkcols = yo[:, b * C : (b + 1) * C]
        blk0 = yo0[:, b * SH * C : (b + 1) * SH * C]
        for a in atoms:
            am = int(a % RING)
            y[a] += blkcols[rows + am, :]
            y[a] += blk0[am].reshape(SH, C)
    return y


def run_kernel(inputs, trace=False):
    from concourse.bass_utils import run_bass_kernel_spmd

    nb, in_maps, merge_plan = _prep(
        inputs["x"],
        inputs["radial_ij"],
        inputs["dir_ij"],
        inputs["cutoff_ij"],
        inputs["Wf"],
        inputs["bf"],
        inputs["idx_i"],
        inputs["idx_j"],
    )
    nc = _get_program(nb)
    res = run_bass_kernel_spmd(nc, in_maps, list(range(NCORES)), trace=trace)
    y = _merge(res.results, merge_plan, nb)
    return y, res


def kernel(**inputs) -> np.ndarray:
    y, _ = run_kernel(inputs, trace=False)
    return y
